# revision 34
# baseline (speedup 1.0000x reference)
"""Multi-head attention (B=2, T=2048, D=1024, H=16) on 8 Trainium2 NeuronCores.

Sharding: tensor-parallel over heads — core c owns global heads {2c, 2c+1} for
both batch elements (Wq/Wk/Wv column-split, Wo row-split, relpos_bias split
along H).  Each core computes a partial [B, D, T] output-projection product;
the host sums the 8 partials and transposes back to [B, T, D].  SPMD: one
program, per-core weight/relpos slices in the input maps; no collectives.

Device-side layout ("transposed flash attention"): scores are computed as
S^T[k, q] so the exp'd scores are already in the right layout (k on
partitions) to be the moving operand of the P@V matmul — the attention
matrix is never transposed on device.

Design notes:
  - fp16 matmuls everywhere (1 cyc/col on PE, ~8x more mantissa than bf16):
    final absmax error ~2e-3 (rel ~6e-4) vs the fp32 reference.
  - relpos bias is added into the scores PSUM accumulation by an
    identity-stationary fp8 matmul (a DVE tensor_tensor add would run at
    1x mode and dominate).  The causal mask is baked into relposT on the
    host as -240 (exp(S-240) underflows to exactly 0; fp8e4 can't carry
    -1e30).  Fully-masked k-blocks are skipped and diagonal-band blocks are
    column-restricted to the causal wavefront (~38% less attention work).
  - key-pad mask rides the ACT exp instruction as a per-partition bias.
  - softmax max-subtraction is skipped (scores are O(+-10), exp is safe in
    fp32); the denominator comes free as an extra row of the P@V matmul
    from an all-ones column appended to V; 1/sqrt(dk) is folded into Wq.
  - normalization (1/denom broadcast along partitions) uses
    reciprocal_approx_fast (SBUF source only — it silently corrupts from
    PSUM, and single-partition slices at base 64 return zeros) and an
    exact hi/lo-fp16 ones-outer-product matmul pair.
  - the whole program is emitted as one software-pipelined stream: batch-1
    projections, per-q-group normalizations, and all output-projection
    pieces are interleaved into the attention k-loops so the PE never
    idles long enough to re-throttle (HAM) and DMA stays behind compute.
  - weight-stationary emission order (all S, all relpos-add, all P@V per
    k-chunk) keeps same-weight matmuls back-to-back.
"""

import sys

for p in ("/opt/trn_rl_repo", "/root/.axon_site/_ro/trn_rl_repo"):
    if p not in sys.path:
        sys.path.insert(0, p)

import numpy as np
import ml_dtypes

import concourse.bacc as bacc
import concourse.mybir as mybir
import concourse.tile as tile
from concourse.bass_utils import run_bass_kernel_spmd

B, T, D, H = 2, 2048, 1024, 16
DK = D // H          # 64
NCORES = 8
HPC = H // NCORES    # heads per core = 2
QG = 512             # q-group width
NQG = T // QG        # 4
NKC = T // 128       # 16 k-chunks
NDC = D // 128       # 8 d-chunks
NEG = np.float32(-1e30)

F32 = mybir.dt.float32
FP16 = mybir.dt.float16
FP8 = mybir.dt.float8e4

_CACHE = {}


def _build_program():
    nc = bacc.Bacc("TRN2", target_bir_lowering=False, debug=False,
                   enable_asserts=True)

    d_qT = nc.dram_tensor("qT", [B, D, T], FP16, kind="ExternalInput").ap()
    d_kT = nc.dram_tensor("kT", [B, D, T], FP16, kind="ExternalInput").ap()
    d_vT = nc.dram_tensor("vT", [B, D, T], FP16, kind="ExternalInput").ap()
    d_rp = nc.dram_tensor("relposT", [HPC, T, T], FP8, kind="ExternalInput").ap()
    d_kp = nc.dram_tensor("kpadT", [128, B, NKC], F32, kind="ExternalInput").ap()
    d_wq = nc.dram_tensor("wqT", [128, NDC, 128], FP16, kind="ExternalInput").ap()
    d_wk = nc.dram_tensor("wkT", [128, NDC, 128], FP16, kind="ExternalInput").ap()
    d_wv = nc.dram_tensor("wvT", [128, NDC, 128], FP16, kind="ExternalInput").ap()
    d_wo = nc.dram_tensor("woT", [HPC, DK, D], FP16, kind="ExternalInput").ap()
    d_id8 = nc.dram_tensor("id8", [128, 128], FP8, kind="ExternalInput").ap()
    d_out = nc.dram_tensor("outT", [B, D, T], FP16, kind="ExternalOutput").ap()

    with tile.TileContext(nc) as tc:
        with (
            tc.tile_pool(name="persist", bufs=1) as persist,
            tc.tile_pool(name="stream", bufs=6) as stream,
            tc.tile_pool(name="rp", bufs=6) as rppool,
            tc.tile_pool(name="ee", bufs=5) as epool,
            tc.tile_pool(name="oc", bufs=6) as ocpool,
            tc.tile_pool(name="nrm", bufs=2) as nrm,
            tc.tile_pool(name="ps", bufs=4, space="PSUM") as ps,
            tc.tile_pool(name="opsum", bufs=4, space="PSUM") as ops,
        ):
            # ---- constants ----
            id8 = persist.tile([128, 128], FP8, tag="id8", name="id8")
            nc.scalar.dma_start(out=id8[:], in_=d_id8[:])
            kpad = persist.tile([128, B, NKC], F32, tag="kpad", name="kpad")
            nc.scalar.dma_start(out=kpad[:], in_=d_kp[:])
            ones = persist.tile([128, DK], F32, tag="ones", name="ones")
            nc.vector.memset(ones[:], 1.0)
            ones16 = persist.tile([128, DK], FP16, tag="ones16", name="ones16")
            nc.vector.memset(ones16[:], 1.0)

            w_sb = {}
            for nm, dten in (("q", d_wq), ("k", d_wk), ("v", d_wv)):
                w = persist.tile([128, NDC, 128], FP16, tag=f"w{nm}",
                                 name=f"w{nm}")
                # host pre-packs [128, NDC, 128] partition-major: one wide
                # contiguous DMA instead of 1024 256B descriptors
                nc.scalar.dma_start(out=w[:], in_=dten[:])
                w_sb[nm] = w
            wo_sb = []
            for h in range(HPC):
                w = persist.tile([DK, D], FP16, tag=f"wo{h}", name=f"wo{h}")
                nc.scalar.dma_start(out=w[:], in_=d_wo[h])
                wo_sb.append(w)

            qt_sb, kt_sb = {}, {}
            vaug = {}
            for b in range(B):
                qt_sb[b] = persist.tile([128, T], FP16, tag=f"qt{b}",
                                        name=f"qt{b}")
                kt_sb[b] = persist.tile([128, T], FP16, tag=f"kt{b}",
                                        name=f"kt{b}")
                for h in range(HPC):
                    va = persist.tile([128, NKC * 80], FP16, tag=f"va{b}{h}",
                                      name=f"va{b}{h}")
                    va_c = va[:].rearrange("p (c u) -> p c u", u=80)
                    nc.vector.tensor_copy(va_c[:, :, 64], ones[:, 0:NKC])
                    vaug[(b, h)] = va

            # ---- projection helpers (weight-stationary, dk outer) ----
            def load_x(dten, b, dk, eng, tag="xin", bufs=10):
                t = stream.tile([128, T], FP16, tag=tag, bufs=bufs,
                                name=f"x{tag}{b}{dk}")
                eng.dma_start(out=t[:],
                              in_=dten[b, dk * 128:(dk + 1) * 128, :])
                return t

            def proj_qk(nm, b, xts, dst, ccs=None):
                ccs = tuple(range(NQG)) if ccs is None else ccs
                accs = {cc: ps.tile([128, QG], F32, tag="ps", name="ps")
                        for cc in ccs}
                for dk in range(NDC):
                    for cc in ccs:
                        nc.tensor.matmul(
                            accs[cc][:], w_sb[nm][:, dk, :],
                            xts[dk][:, cc * QG:(cc + 1) * QG],
                            start=(dk == 0), stop=(dk == NDC - 1))
                for cc in ccs:
                    nc.vector.tensor_copy(
                        dst[b][:, cc * QG:(cc + 1) * QG], accs[cc][:])

            def proj_v(b, vts, tbs=None):
                for tb in (range(NKC) if tbs is None else tbs):
                    ts_ = slice(tb * 128, (tb + 1) * 128)
                    acc = ps.tile([128, 128], F32, tag="ps", name="psv")
                    for dk in range(NDC):
                        nc.tensor.matmul(
                            acc[:], vts[dk][:, ts_], w_sb["v"][:, dk, :],
                            start=(dk == 0), stop=(dk == NDC - 1))
                    for h in range(HPC):
                        nc.vector.tensor_copy(
                            vaug[(b, h)][:, tb * 80:tb * 80 + DK],
                            acc[:, h * DK:(h + 1) * DK])

            # ---- phase 1: batch 0 projections; loads split across rings ----
            engs = (nc.scalar, nc.sync)
            xq0 = {dk: load_x(d_qT, 0, dk, engs[dk % 2]) for dk in range(NDC)}
            vch0 = {dk: load_x(d_vT, 0, dk, engs[dk % 2], "xinv", 8)
                    for dk in range(NDC)}
            proj_qk("q", 0, xq0, qt_sb)
            xk0 = {dk: load_x(d_kT, 0, dk, engs[dk % 2]) for dk in range(NDC)}
            # only the first halves of k/v projections gate attention start;
            # the rest interleave into unit (0,0)'s k-loop below
            proj_v(0, vch0, range(0, 8))
            proj_qk("k", 0, xk0, kt_sb, ccs=(0, 1))

            # ---- phase 2: attention; batch-1 projections and all output
            # projections are interleaved into the instruction stream ----
            at_sb = {}
            for b in range(B):
                for h in range(HPC):
                    at_sb[(b, h)] = persist.tile([DK, T], FP16,
                                                 tag=f"at{b}{h}",
                                                 name=f"at{b}{h}")

            def norm_dve(u, qg):
                o = nrm.tile([DK + 1, QG], F32, tag="oc2", name="oc2")
                nc.vector.tensor_copy(o[:], pend_ops[u][qg][:])
                rc = nrm.tile([DK + 1, QG], F32, tag="rc", name="rc")
                nc.vector.reciprocal_approx_fast(out=rc[:], in_=o[:])
                rch = nrm.tile([DK + 1, QG], FP16, tag="rch", name="rch")
                nc.vector.tensor_copy(rch[:], rc[:])
                rcl = nrm.tile([DK + 1, QG], FP16, tag="rcl", name="rcl")
                nc.vector.tensor_sub(rcl[:], rc[:], rch[:])
                pend_dve[(u, qg)] = (o, rch, rcl)

            def norm_pe(u, qg):
                b, h = u
                o, rch, rcl = pend_dve.pop((u, qg))
                rb = ops.tile([DK, QG], F32, tag="ops", name="rb")
                nc.tensor.matmul(
                    rb[:], ones16[DK:DK + 1, :], rch[DK:DK + 1, :],
                    start=True, stop=False)
                nc.tensor.matmul(
                    rb[:], ones16[DK:DK + 1, :], rcl[DK:DK + 1, :],
                    start=False, stop=True)
                nc.vector.tensor_mul(
                    at_sb[(b, h)][:, qg * QG:(qg + 1) * QG], o[0:DK, :], rb[:])
                # h==1 is always normed after h==0 for a given (b, qg), so
                # at_sb is complete for this qg: queue its out-proj pieces
                # now (a full unit earlier than end-of-batch queueing; the
                # old trailing ~26-piece drain ran HAM-cold for ~35us)
                if h == HPC - 1:
                    oproj_q.extend((b, db, qg) for db in range(NDC))

            def oproj_piece(b, db, qg):
                ds_ = slice(db * 128, (db + 1) * 128)
                qs = slice(qg * QG, (qg + 1) * QG)
                pp = ps.tile([128, QG], F32, tag="ps", name="pp")
                for h in range(HPC):
                    nc.tensor.matmul(
                        pp[:], wo_sb[h][:, ds_], at_sb[(b, h)][:, qs],
                        start=(h == 0), stop=(h == HPC - 1))
                oc = ocpool.tile([128, QG], FP16, tag="oc", name="oc")
                nc.vector.tensor_copy(oc[:], pp[:])
                nc.sync.dma_start(out=d_out[b, ds_, qs], in_=oc[:])

            pend_ops, pend_dve = {}, {}
            pending = []          # (unit, qg) whose norm_pe is still owed
            oproj_q = []          # (b, db, qg) out-proj pieces ready to emit
            units = [(b, h) for b in range(B) for h in range(HPC)]
            last_u = units[-1]
            xq1 = vch1 = None
            for ui, u in enumerate(units):
                b, h = u
                hs = slice(h * DK, (h + 1) * DK)
                if ui == 0:
                    # prefetch batch-1 q and v rows.  These issues can wait
                    # minutes-long (in engine terms) on pool-slot semaphores,
                    # so they MUST NOT sit in the Scalar queue where they
                    # would stall the exp instructions behind them; the
                    # otherwise-idle GpSimd (SWDGE) queue absorbs the waits.
                    xq1 = {dk: load_x(d_qT, 1, dk, nc.gpsimd)
                           for dk in range(NDC)}
                    vch1 = {dk: load_x(d_vT, 1, dk, nc.gpsimd, "xinv", 8)
                            for dk in range(NDC)}
                if ui == 1:
                    # batch-1 projections: data already resident
                    proj_qk("q", 1, xq1, qt_sb)
                    xk1 = {dk: load_x(d_kT, 1, dk, nc.gpsimd)
                           for dk in range(NDC)}
                    proj_v(1, vch1)
                    proj_qk("k", 1, xk1, kt_sb)
                pend_ops[u] = [ops.tile([DK + 1, QG], F32, tag="ops",
                                        name="ops") for _ in range(NQG)]
                o_ps = pend_ops[u]
                for ck in range(NKC):
                    if ck == 4 and pending:
                        norm_pe(*pending.pop(0))
                    if ck >= 5 and (ck - 5) % 4 == 0:
                        qgn = (ck - 5) // 4
                        norm_pe(u, qgn)
                    if ck >= 3 and oproj_q:
                        oproj_piece(*oproj_q.pop(0))
                        if len(oproj_q) > 6:
                            oproj_piece(*oproj_q.pop(0))
                    if ui == 0:
                        # deferred second halves of the batch-0 projections
                        # (their x chunks land while attention runs)
                        if ck == 1:
                            proj_qk("k", 0, xk0, kt_sb, ccs=(2,))
                        elif ck == 2:
                            proj_qk("k", 0, xk0, kt_sb, ccs=(3,))
                        elif ck == 3:
                            proj_v(0, vch0, range(8, 12))
                        elif ck == 5:
                            proj_v(0, vch0, range(12, 16))
                    qg0 = ck // 4          # first valid q-group
                    off = qg0 * QG         # start col of rp tile
                    rp_t = rppool.tile([128, T], FP8, tag="rp", name="rp")
                    nc.sync.dma_start(
                        out=rp_t[:, 0:T - off],
                        in_=d_rp[h, ck * 128:(ck + 1) * 128, off:T])
                    s_ts, cos, ws = [], [], []
                    for qg in range(qg0, NQG):
                        co = max(0, ck * 128 - qg * QG)
                        w = QG - co
                        cos.append(co)
                        ws.append(w)
                        s_t = ps.tile([128, QG], F32, tag="ps", name="ps")
                        s_ts.append(s_t)
                        nc.tensor.matmul(
                            s_t[:, 0:w],
                            kt_sb[b][hs, ck * 128:(ck + 1) * 128],
                            qt_sb[b][hs, qg * QG + co:(qg + 1) * QG],
                            start=True, stop=False)
                    for i, qg in enumerate(range(qg0, NQG)):
                        rj = qg * QG + cos[i] - off
                        nc.tensor.matmul(
                            s_ts[i][:, 0:ws[i]], id8[:],
                            rp_t[:, rj:rj + ws[i]],
                            start=False, stop=True)
                    e_ts = []
                    for i, qg in enumerate(range(qg0, NQG)):
                        e_t = epool.tile([128, QG], FP16, tag="ee", name="ee")
                        e_ts.append(e_t)
                        nc.scalar.activation(
                            e_t[:, 0:ws[i]], s_ts[i][:, 0:ws[i]],
                            mybir.ActivationFunctionType.Exp,
                            bias=kpad[:, b, ck:ck + 1])
                    for i, qg in enumerate(range(qg0, NQG)):
                        nc.tensor.matmul(
                            o_ps[qg][:, cos[i]:QG],
                            vaug[(b, h)][:, ck * 80:ck * 80 + DK + 1],
                            e_ts[i][:, 0:ws[i]],
                            start=(ck == 0), stop=(ck == 4 * qg + 3))
                    if ck % 4 == 3:
                        norm_dve(u, ck // 4)
                pending.append((u, NQG - 1))
            while pending:
                norm_pe(*pending.pop(0))
            while oproj_q:
                oproj_piece(*oproj_q.pop(0))

    nc.compile()
    return nc


def _prep_host(q, k, v, key_pad_mask, attn_mask, relpos_bias, Wq, Wk, Wv, Wo):
    f32, f16 = np.float32, np.float16
    qT = np.asarray(q, f32).transpose(0, 2, 1).astype(f16)
    kT = np.asarray(k, f32).transpose(0, 2, 1).astype(f16)
    vT = np.asarray(v, f32).transpose(0, 2, 1).astype(f16)

    kb = np.where(np.asarray(key_pad_mask), NEG, f32(0)).astype(f32)  # [B,T]
    kpadT = np.ascontiguousarray(kb.reshape(B, NKC, 128).transpose(2, 0, 1))

    maskT = np.asarray(attn_mask).T  # [k, q], True = masked (k > q)
    rp = np.asarray(relpos_bias, f32)

    id8 = np.eye(128, dtype=ml_dtypes.float8_e4m3)

    def _wmajor(W, rows):  # this core's 128 out-dims -> [128, NDC, 128]
        wT = np.ascontiguousarray(W[rows].T)  # [D, 128]
        return np.ascontiguousarray(
            wT.reshape(NDC, 128, 128).transpose(1, 0, 2)).astype(f16)

    Wq = np.asarray(Wq, f32) * f32(1.0 / np.sqrt(DK))
    Wk = np.asarray(Wk, f32)
    Wv = np.asarray(Wv, f32)
    Wo = np.asarray(Wo, f32)

    in_maps = []
    for c in range(NCORES):
        rows = slice(c * 128, (c + 1) * 128)
        h0 = 2 * c
        rpT = np.where(maskT[None], f32(-240.0),
                       rp[h0:h0 + 2].transpose(0, 2, 1)).astype(
                           ml_dtypes.float8_e4m3)
        woT = np.ascontiguousarray(
            np.stack([Wo[:, (2 * c + h) * DK:(2 * c + h + 1) * DK].T
                      for h in range(HPC)])).astype(f16)
        in_maps.append({
            "qT": qT, "kT": kT, "vT": vT,
            "relposT": np.ascontiguousarray(rpT),
            "kpadT": kpadT,
            "wqT": _wmajor(Wq, rows),
            "wkT": _wmajor(Wk, rows),
            "wvT": _wmajor(Wv, rows),
            "woT": woT,
            "id8": id8,
        })
    return in_maps


def run(trace=False, tmpdir=None, **inputs):
    if "nc" not in _CACHE:
        _CACHE["nc"] = _build_program()
    nc = _CACHE["nc"]
    in_maps = _prep_host(**inputs)
    res = run_bass_kernel_spmd(nc, in_maps, core_ids=list(range(NCORES)),
                               trace=trace, tmpdir=tmpdir)
    acc = res.results[0]["outT"].astype(np.float64)
    for c in range(1, NCORES):
        acc += res.results[c]["outT"]
    out = np.ascontiguousarray(acc.transpose(0, 2, 1)).astype(np.float32)
    return out, res


def kernel(**inputs) -> np.ndarray:
    out, _ = run(trace=False, **inputs)
    return out

